# revision 16
# baseline (speedup 1.0000x reference)
"""ChatGLM self-attention (MQA, rotary, causal) on 8 TRN2 NeuronCores.

Sharding: tensor-parallel over heads. Core c computes Q-heads [4c, 4c+4)
and the KV group g=c//4 it needs. Dense is row-parallel; the 8 partial
outputs (bf16) are summed on host (the RowParallel unshard).

Key device-side structure (everything channel-major, mixed^T):
- QKV projection and the output dense run as fp8-e4m3 DoubleRow matmuls
  (256-deep contraction per instruction at 0.5 cyc/row) on a hi+lo
  split: x*w ~= x_hi*w_hi + x_hi*w_lo + x_lo*w_hi, dropping only the
  lo*lo term (~1e-3 relative). Weights and hidden states are split on
  the host; ctx is split on-device after the 1/l scaling.
- Attention S^T = K^T.T @ Q^T in bf16, exp on Act -> P bf16,
  ctx^T = V_tm.T @ P^T accumulated in PSUM. Softmax denominator: P
  tiles are summed on DVE (bf16, 2x mode) into P_sum and contracted
  with a ones-vector in ONE [1,512]-out matmul per (b,chunk,head).
- Q^T stays resident in SBUF between the phases (no DRAM round trip).
- Diagonal (causal) tiles use narrowed moving slices.
- W_qkv columns are permuted on host so rotary pairs become contiguous
  partition blocks (evens 0:32, odds 32:64, pass-through 64:128).
"""

import numpy as np
import ml_dtypes

import concourse.bass as bass
import concourse.tile as tile
from concourse import bacc, mybir
from concourse.bass_utils import run_bass_kernel_spmd
from concourse.masks import make_identity

F32 = mybir.dt.float32
F32R = mybir.dt.float32r
BF16 = mybir.dt.bfloat16
F8 = mybir.dt.float8e4
AF = mybir.ActivationFunctionType
DR = mybir.MatmulPerfMode.DoubleRow

N_CORES = 8
SQ, B, H = 2048, 2, 4096
NH, HD = 32, 128
NG = 2
ROT = 64
HPC = NH // N_CORES          # heads per core = 4
QCOLS = HPC * HD             # 512
CCOLS = QCOLS + 2 * HD       # 768: Q(512) K(128) V(128)
NCT = CCOLS // 128           # 6 c-tiles
TOK = SQ * B                 # 4096
CHUNK = 512
NCHUNK = TOK // CHUNK        # 8
NP2 = H // 256               # 16 channel-pair tiles
SCALE = 1.0 / float(np.sqrt(HD))
WSCALE = 64.0                # lift sigma~0.02 weights out of fp8 subnormals

_CACHE: dict = {}


def _build():
    nc = bacc.Bacc(None, target_bir_lowering=False, num_devices=N_CORES)

    hid_hi = nc.dram_tensor("hid_hi", [128, NP2, 2, TOK], F8, kind="ExternalInput")
    hid_lo = nc.dram_tensor("hid_lo", [128, NP2, 2, TOK], F8, kind="ExternalInput")
    wq_hi = nc.dram_tensor("wq_hi", [128, NP2, 2, CCOLS], F8, kind="ExternalInput")
    wq_lo = nc.dram_tensor("wq_lo", [128, NP2, 2, CCOLS], F8, kind="ExternalInput")
    bq = nc.dram_tensor("bq", [128, NCT], F32, kind="ExternalInput")
    wd_hi = nc.dram_tensor("wd_hi", [128, 2, 2, H], F8, kind="ExternalInput")
    wd_lo = nc.dram_tensor("wd_lo", [128, 2, 2, H], F8, kind="ExternalInput")
    cos128 = nc.dram_tensor("cos128", [128, SQ], BF16, kind="ExternalInput")
    snpm = nc.dram_tensor("snpm", [64, SQ], BF16, kind="ExternalInput")
    maskd = nc.dram_tensor("maskd", [128, 128], BF16, kind="ExternalInput")
    ones_col = nc.dram_tensor("ones_col", [128, 1], BF16, kind="ExternalInput")
    ones_row = nc.dram_tensor("ones_row", [1, 128], F32, kind="ExternalInput")
    out_p = nc.dram_tensor("out_p", [TOK, H], BF16, kind="ExternalOutput")

    with tile.TileContext(nc) as tc:
        with (
            nc.allow_low_precision(reason="bf16/fp8 pipeline, tolerance 2e-2"),
            tc.tile_pool(name="persist", bufs=1) as persist,
        ):
            qT = persist.tile([128, HPC, TOK], BF16)      # Q^T, d-major, resident
            kT = persist.tile([128, B, SQ], BF16)         # K^T, d-major
            v_tm = persist.tile([128, B, SQ // 128, 128], BF16)  # V token-major
            wd_hi_t = persist.tile([128, 2, 2, H], F8)
            wd_lo_t = persist.tile([128, 2, 2, H], F8)
            bq_t = persist.tile([128, NCT], F32)
            mask_t = persist.tile([128, 128], BF16)
            onec = persist.tile([128, 1], BF16)
            oner = persist.tile([1, 128], F32R)
            ident = persist.tile([128, 128], BF16)
            cos_t = persist.tile([128, SQ], BF16)
            sin_t = persist.tile([64, SQ], BF16)

            # everything but hid/out DMAs goes on the Act queue so the
            # first hid-chunk DMAs (SP) reach the DMA engines immediately
            nc.scalar.dma_start(bq_t[:], bq[:])
            make_identity(nc, ident[:])

            # ---------- phase 1: QKV projection (fp8 DR) + rotary ----------
            with (
                tc.tile_pool(name="p1w", bufs=1) as p1w,
                tc.tile_pool(name="p1hid", bufs=4) as p1hid,
                tc.tile_pool(name="p1", bufs=4) as p1,
                tc.tile_pool(name="p1ps", bufs=NCT + 1, space="PSUM") as p1ps,
                tc.tile_pool(name="p1tps", bufs=1, space="PSUM") as p1tps,
            ):
                wqh = p1w.tile([128, NP2, 2, CCOLS], F8)
                wql = p1w.tile([128, NP2, 2, CCOLS], F8)
                for pi in range(NP2):
                    nc.scalar.dma_start(wqh[:, pi], wq_hi[:, pi])
                    nc.scalar.dma_start(wql[:, pi], wq_lo[:, pi])
                nc.scalar.dma_start(cos_t[:], cos128[:])
                nc.scalar.dma_start(sin_t[:], snpm[:])
                nc.scalar.dma_start(mask_t[:], maskd[:])
                nc.scalar.dma_start(onec[:], ones_col[:])
                nc.scalar.dma_start(oner[:], ones_row[:].bitcast(F32R))
                for hp in range(2):
                    nc.scalar.dma_start(wd_hi_t[:, hp], wd_hi[:, hp])
                    nc.scalar.dma_start(wd_lo_t[:, hp], wd_lo[:, hp])

                for tcn in range(NCHUNK):
                    b = tcn // (SQ // CHUNK)
                    s0 = (tcn % (SQ // CHUNK)) * CHUNK
                    t0 = tcn * CHUNK
                    pss = [
                        p1ps.tile([128, CHUNK], F32, tag="qkvps",
                                  name=f"qkvps{ct}")
                        for ct in range(NCT)
                    ]
                    for pg in range(4):
                        hh = p1hid.tile([128, 4, 2, CHUNK], F8, tag="hh")
                        nc.sync.dma_start(
                            hh[:], hid_hi[:, pg * 4:(pg + 1) * 4, :, t0:t0 + CHUNK])
                        hl = p1hid.tile([128, 4, 2, CHUNK], F8, tag="hl")
                        nc.sync.dma_start(
                            hl[:], hid_lo[:, pg * 4:(pg + 1) * 4, :, t0:t0 + CHUNK])
                        for pl in range(4):
                            pi = pg * 4 + pl
                            for ct in range(NCT):
                                wh = wqh[:, pi, :, ct * 128:(ct + 1) * 128]
                                wl = wql[:, pi, :, ct * 128:(ct + 1) * 128]
                                nc.tensor.matmul(
                                    pss[ct][:], wh, hh[:, pl], perf_mode=DR,
                                    start=(pi == 0), stop=False)
                                nc.tensor.matmul(
                                    pss[ct][:], wl, hh[:, pl], perf_mode=DR,
                                    start=False, stop=False)
                                nc.tensor.matmul(
                                    pss[ct][:], wh, hl[:, pl], perf_mode=DR,
                                    start=False, stop=(pi == NP2 - 1))

                    cs = cos_t[:, s0:s0 + CHUNK]
                    sn = sin_t[:, s0:s0 + CHUNK]
                    # Q heads + K: bias, then rotary in bf16
                    for ct in range(HPC + 1):
                        mix = p1.tile([128, CHUNK], BF16, tag="mix")
                        nc.scalar.activation(
                            mix[:], pss[ct][:], AF.Identity,
                            bias=bq_t[:, ct:ct + 1], scale=1.0 / WSCALE)
                        if ct < HPC:
                            dst = qT[:, ct, t0:t0 + CHUNK]
                        else:
                            dst = kT[:, b, s0:s0 + CHUNK]
                        swp = p1.tile([64, CHUNK], BF16, tag="swp")
                        nc.sync.dma_start(swp[0:32], mix[32:64])
                        nc.sync.dma_start(swp[32:64], mix[0:32])
                        nc.vector.tensor_mul(out=dst, in0=mix[:], in1=cs)
                        nc.vector.tensor_mul(out=swp[:], in0=swp[:], in1=sn)
                        nc.vector.tensor_add(
                            out=dst[0:64], in0=dst[0:64], in1=swp[:])
                    # V: bias then transpose to token-major
                    mixv = p1.tile([128, CHUNK], BF16, tag="mixv")
                    nc.scalar.activation(
                        mixv[:], pss[HPC + 1][:], AF.Identity,
                        bias=bq_t[:, HPC + 1:HPC + 2], scale=1.0 / WSCALE)
                    for q4 in range(CHUNK // 128):
                        tps = p1tps.tile([128, 128], BF16, tag="tps")
                        nc.tensor.transpose(
                            tps[:], mixv[:, q4 * 128:(q4 + 1) * 128], ident[:])
                        nc.vector.tensor_copy(
                            out=v_tm[:, b, s0 // 128 + q4, :], in_=tps[:])

            # ---------- phase 2: attention + dense ----------
            with (
                tc.tile_pool(name="p2", bufs=4) as p2,
                tc.tile_pool(name="p2p", bufs=3) as p2p,
                tc.tile_pool(name="p2ctx", bufs=2) as p2ctx,
                tc.tile_pool(name="p2osb", bufs=2) as p2osb,
                tc.tile_pool(name="p2sps", bufs=2, space="PSUM") as p2sps,
                tc.tile_pool(name="p2cps", bufs=2, space="PSUM") as p2cps,
                tc.tile_pool(name="p2lps", bufs=1, space="PSUM") as p2lps,
                tc.tile_pool(name="p2bps", bufs=1, space="PSUM") as p2bps,
                tc.tile_pool(name="p2dps", bufs=2, space="PSUM") as p2dps,
            ):
                def emit_scale(linv, ctx_ps, h, chi, clo):
                    # 1/l broadcast + ctx scale + fp8 hi/lo split; emitted one
                    # head late so PE never waits on the DVE reciprocal
                    lb_ps = p2bps.tile([128, CHUNK], F32, tag="lbps")
                    nc.tensor.matmul(
                        lb_ps[:], oner[:], linv[:], start=True, stop=True)
                    lb_sb = p2.tile([128, CHUNK], F32, tag="lbsb")
                    nc.vector.tensor_copy(out=lb_sb[:], in_=lb_ps[:])
                    ctxT = p2.tile([128, CHUNK], F32, tag="ctxT")
                    nc.vector.tensor_mul(
                        out=ctxT[:], in0=ctx_ps[:], in1=lb_sb[:])
                    hp, hx = h // 2, h % 2
                    nc.scalar.activation(
                        chi[hp][:, hx, :], ctxT[:], AF.Identity)
                    nc.vector.tensor_sub(
                        out=clo[hp][:, hx, :], in0=ctxT[:],
                        in1=chi[hp][:, hx, :])

                for b in range(B):
                    for sc in range(SQ // CHUNK):
                        base = b * SQ + sc * CHUNK
                        n_t = (sc + 1) * (CHUNK // 128)
                        chi = [p2ctx.tile([128, 2, CHUNK], F8, tag="chi",
                                          name=f"chi{hp}") for hp in range(2)]
                        clo = [p2ctx.tile([128, 2, CHUNK], F8, tag="clo",
                                          name=f"clo{hp}") for hp in range(2)]
                        # two in-flight l vectors share one PSUM bank
                        # (partition rows 0 and 32)
                        lpair = p2lps.tile([33, CHUNK], F32, tag="lpair")
                        pend = None
                        for h in range(HPC):
                            ctx_ps = p2cps.tile([128, CHUNK], F32, tag="ctxps")
                            p_sum = p2p.tile([128, CHUNK], BF16, tag="psum")
                            for tt in range(n_t):
                                j = tt - sc * (CHUNK // 128)
                                so = 128 * j if j >= 0 else 0
                                sp = p2sps.tile([128, CHUNK], F32, tag="sps")
                                nc.tensor.matmul(
                                    sp[:, so:], kT[:, b, tt * 128:(tt + 1) * 128],
                                    qT[:, h, base + so:base + CHUNK],
                                    start=True, stop=True)
                                p_r = p2p.tile([128, CHUNK], BF16, tag="p")
                                nc.scalar.activation(
                                    p_r[:, so:], sp[:, so:], AF.Exp, scale=SCALE)
                                if j >= 0:
                                    nc.vector.tensor_mul(
                                        out=p_r[:, so:so + 128],
                                        in0=p_r[:, so:so + 128], in1=mask_t[:])
                                if tt == 0:
                                    nc.vector.tensor_copy(
                                        out=p_sum[:], in_=p_r[:])
                                else:
                                    nc.vector.tensor_add(
                                        out=p_sum[:, so:], in0=p_sum[:, so:],
                                        in1=p_r[:, so:])
                                nc.tensor.matmul(
                                    ctx_ps[:, so:], v_tm[:, b, tt, :],
                                    p_r[:, so:], start=(tt == 0),
                                    stop=(tt == n_t - 1),
                                    skip_group_check=True)
                            lrow = 32 * (h % 2)
                            l_out = lpair[lrow:lrow + 1, :]
                            nc.tensor.matmul(
                                l_out, onec[:], p_sum[:], start=True,
                                stop=True)
                            linv = p2.tile([1, CHUNK], F32R, tag="linv")
                            nc.vector.reciprocal(linv[:], l_out)
                            if pend is not None:
                                emit_scale(*pend, chi, clo)
                            pend = (linv, ctx_ps, h)
                        emit_scale(*pend, chi, clo)

                        row0 = b * SQ + sc * CHUNK
                        for st in range(CHUNK // 128):
                            osb = p2osb.tile([128, H // CHUNK, CHUNK], BF16,
                                             tag="osb")
                            for oc in range(H // CHUNK):
                                dps = p2dps.tile([128, CHUNK], F32, tag="dps")
                                for hp in range(2):
                                    cH = chi[hp][:, :, st * 128:(st + 1) * 128]
                                    cL = clo[hp][:, :, st * 128:(st + 1) * 128]
                                    wH = wd_hi_t[:, hp, :,
                                                 oc * CHUNK:(oc + 1) * CHUNK]
                                    wL = wd_lo_t[:, hp, :,
                                                 oc * CHUNK:(oc + 1) * CHUNK]
                                    nc.tensor.matmul(
                                        dps[:], cH, wH, perf_mode=DR,
                                        start=(hp == 0), stop=False)
                                    nc.tensor.matmul(
                                        dps[:], cH, wL, perf_mode=DR,
                                        start=False, stop=False)
                                    nc.tensor.matmul(
                                        dps[:], cL, wH, perf_mode=DR,
                                        start=False, stop=(hp == 1))
                                if oc % 2 == 0:
                                    nc.scalar.activation(
                                        osb[:, oc, :], dps[:], AF.Identity)
                                else:
                                    nc.vector.tensor_copy(
                                        out=osb[:, oc, :], in_=dps[:])
                            nc.sync.dma_start(
                                out_p[row0 + st * 128:row0 + (st + 1) * 128, :],
                                osb[:])

    nc.compile()
    return nc


def _split8(x):
    hi = x.astype(ml_dtypes.float8_e4m3)
    lo = (x - hi.astype(np.float32)).astype(ml_dtypes.float8_e4m3)
    return hi, lo


def _pair_layout(x, np2):
    # [np2*256, n] -> [128, np2, 2, n]
    n = x.shape[1]
    return np.ascontiguousarray(
        x.reshape(np2, 2, 128, n).transpose(2, 0, 1, 3))


def _host_inputs(hidden_states, rotary_pos_emb, W_qkv, b_qkv, W_dense):
    hidden_states = np.asarray(hidden_states, dtype=np.float32)
    rope = np.asarray(rotary_pos_emb, dtype=np.float32)
    W_qkv = np.asarray(W_qkv, dtype=np.float32)
    b_qkv = np.asarray(b_qkv, dtype=np.float32)
    W_dense = np.asarray(W_dense, dtype=np.float32)

    hidT = np.ascontiguousarray(
        hidden_states.transpose(2, 1, 0).reshape(H, TOK))
    h_hi, h_lo = _split8(hidT)
    hid_hi = _pair_layout(h_hi, NP2)
    hid_lo = _pair_layout(h_lo, NP2)

    cos = rope[:, :, 0].T  # [32, sq]
    sin = rope[:, :, 1].T
    cos128 = np.concatenate(
        [cos, cos, np.ones((64, SQ), np.float32)], axis=0
    ).astype(ml_dtypes.bfloat16)
    snpm = np.concatenate([-sin, sin], axis=0).astype(ml_dtypes.bfloat16)
    maskd = (np.arange(128)[None, :] >= np.arange(128)[:, None]).astype(
        ml_dtypes.bfloat16)
    ones_col = np.ones((128, 1), ml_dtypes.bfloat16)
    ones_row = np.ones((1, 128), np.float32)

    perm = np.concatenate(
        [np.arange(0, ROT, 2), np.arange(1, ROT, 2), np.arange(ROT, HD)])
    in_maps = []
    for c in range(N_CORES):
        g = c // (N_CORES // NG)
        qcols = [h * HD + perm for h in range(HPC * c, HPC * (c + 1))]
        kcols = NH * HD + g * HD + perm
        vcols = NH * HD + NG * HD + g * HD + np.arange(HD)
        cols = np.concatenate(qcols + [kcols, vcols])
        wq_c = np.ascontiguousarray(W_qkv[:, cols]) * WSCALE
        w_hi, w_lo = _split8(wq_c)
        bq_c = np.ascontiguousarray(b_qkv[cols].reshape(NCT, 128).T)
        wd_c = np.ascontiguousarray(W_dense[c * QCOLS:(c + 1) * QCOLS, :]) * WSCALE
        d_hi, d_lo = _split8(wd_c)
        in_maps.append({
            "hid_hi": hid_hi, "hid_lo": hid_lo,
            "wq_hi": _pair_layout(w_hi, NP2), "wq_lo": _pair_layout(w_lo, NP2),
            "bq": bq_c,
            "wd_hi": _pair_layout(d_hi, 2), "wd_lo": _pair_layout(d_lo, 2),
            "cos128": cos128, "snpm": snpm, "maskd": maskd,
            "ones_col": ones_col, "ones_row": ones_row,
        })
    return in_maps


def kernel(hidden_states, attention_mask, rotary_pos_emb, W_qkv, b_qkv,
           W_dense, _trace=False):
    if "nc" not in _CACHE:
        _CACHE["nc"] = _build()
    nc = _CACHE["nc"]
    in_maps = _host_inputs(
        hidden_states, rotary_pos_emb, W_qkv, b_qkv, W_dense)
    res = run_bass_kernel_spmd(
        nc, in_maps, list(range(N_CORES)), trace=_trace)
    acc = res.results[0]["out_p"].astype(np.float32)
    for c in range(1, N_CORES):
        acc += res.results[c]["out_p"].astype(np.float32)
    acc *= 1.0 / WSCALE
    out = acc.reshape(B, SQ, H).transpose(1, 0, 2)
    out = np.ascontiguousarray(out)
    _CACHE["last_result"] = res
    return out


# revision 17
# speedup vs baseline: 1.0016x; 1.0016x over previous
"""ChatGLM self-attention (MQA, rotary, causal) on 8 TRN2 NeuronCores.

Sharding: tensor-parallel over heads. Core c computes Q-heads [4c, 4c+4)
and the KV group g=c//4 it needs. Dense is row-parallel; the 8 partial
outputs (bf16) are summed on host (the RowParallel unshard).

Key device-side structure (everything channel-major, mixed^T):
- QKV projection and the output dense run as fp8-e4m3 DoubleRow matmuls
  (256-deep contraction per instruction at 0.5 cyc/row) on a hi+lo
  split: x*w ~= x_hi*w_hi + x_hi*w_lo + x_lo*w_hi, dropping only the
  lo*lo term (~1e-3 relative). Weights and hidden states are split on
  the host; ctx is split on-device after the 1/l scaling.
- Attention S^T = K^T.T @ Q^T in bf16, exp on Act -> P bf16,
  ctx^T = V_tm.T @ P^T accumulated in PSUM. Softmax denominator: P
  tiles are summed on DVE (bf16, 2x mode) into P_sum and contracted
  with a ones-vector in ONE [1,512]-out matmul per (b,chunk,head).
- Q^T stays resident in SBUF between the phases (no DRAM round trip).
- Diagonal (causal) tiles use narrowed moving slices.
- W_qkv columns are permuted on host so rotary pairs become contiguous
  partition blocks (evens 0:32, odds 32:64, pass-through 64:128).
"""

import numpy as np
import ml_dtypes

import concourse.bass as bass
import concourse.tile as tile
from concourse import bacc, mybir
from concourse.bass_utils import run_bass_kernel_spmd
from concourse.masks import make_identity

F32 = mybir.dt.float32
F32R = mybir.dt.float32r
BF16 = mybir.dt.bfloat16
F8 = mybir.dt.float8e4
AF = mybir.ActivationFunctionType
DR = mybir.MatmulPerfMode.DoubleRow

N_CORES = 8
SQ, B, H = 2048, 2, 4096
NH, HD = 32, 128
NG = 2
ROT = 64
HPC = NH // N_CORES          # heads per core = 4
QCOLS = HPC * HD             # 512
CCOLS = QCOLS + 2 * HD       # 768: Q(512) K(128) V(128)
NCT = CCOLS // 128           # 6 c-tiles
TOK = SQ * B                 # 4096
CHUNK = 512
NCHUNK = TOK // CHUNK        # 8
NP2 = H // 256               # 16 channel-pair tiles
SCALE = 1.0 / float(np.sqrt(HD))
WSCALE = 64.0                # lift sigma~0.02 weights out of fp8 subnormals

_CACHE: dict = {}


def _build():
    nc = bacc.Bacc(None, target_bir_lowering=False, num_devices=N_CORES)

    hid_hi = nc.dram_tensor("hid_hi", [128, NP2, 2, TOK], F8, kind="ExternalInput")
    hid_lo = nc.dram_tensor("hid_lo", [128, NP2, 2, TOK], F8, kind="ExternalInput")
    wq_hi = nc.dram_tensor("wq_hi", [128, NP2, 2, CCOLS], F8, kind="ExternalInput")
    wq_lo = nc.dram_tensor("wq_lo", [128, NP2, 2, CCOLS], F8, kind="ExternalInput")
    bq = nc.dram_tensor("bq", [128, NCT], F32, kind="ExternalInput")
    wd_hi = nc.dram_tensor("wd_hi", [128, 2, 2, H], F8, kind="ExternalInput")
    wd_lo = nc.dram_tensor("wd_lo", [128, 2, 2, H], F8, kind="ExternalInput")
    cos128 = nc.dram_tensor("cos128", [128, SQ], BF16, kind="ExternalInput")
    snpm = nc.dram_tensor("snpm", [64, SQ], BF16, kind="ExternalInput")
    maskd = nc.dram_tensor("maskd", [128, 128], BF16, kind="ExternalInput")
    ones_col = nc.dram_tensor("ones_col", [128, 1], BF16, kind="ExternalInput")
    ones_row = nc.dram_tensor("ones_row", [1, 128], F32, kind="ExternalInput")
    out_p = nc.dram_tensor("out_p", [TOK, H], BF16, kind="ExternalOutput")

    with tile.TileContext(nc) as tc:
        with (
            nc.allow_low_precision(reason="bf16/fp8 pipeline, tolerance 2e-2"),
            tc.tile_pool(name="persist", bufs=1) as persist,
        ):
            qT = persist.tile([128, HPC, TOK], BF16)      # Q^T, d-major, resident
            kT = persist.tile([128, B, SQ], BF16)         # K^T, d-major
            v_tm = persist.tile([128, B, SQ // 128, 128], BF16)  # V token-major
            wd_hi_t = persist.tile([128, 2, 2, H], F8)
            wd_lo_t = persist.tile([128, 2, 2, H], F8)
            bq_t = persist.tile([128, NCT], F32)
            mask_t = persist.tile([128, 128], BF16)
            onec = persist.tile([128, 1], BF16)
            oner = persist.tile([1, 128], F32R)
            ident = persist.tile([128, 128], BF16)
            cos_t = persist.tile([128, SQ], BF16)
            sin_t = persist.tile([64, SQ], BF16)

            # everything but hid/out DMAs goes on the Act queue so the
            # first hid-chunk DMAs (SP) reach the DMA engines immediately
            nc.scalar.dma_start(bq_t[:], bq[:])
            make_identity(nc, ident[:])

            # ---------- phase 1: QKV projection (fp8 DR) + rotary ----------
            with (
                tc.tile_pool(name="p1w", bufs=1) as p1w,
                tc.tile_pool(name="p1hid", bufs=4) as p1hid,
                tc.tile_pool(name="p1", bufs=4) as p1,
                tc.tile_pool(name="p1ps", bufs=NCT + 1, space="PSUM") as p1ps,
                tc.tile_pool(name="p1tps", bufs=1, space="PSUM") as p1tps,
            ):
                wqh = p1w.tile([128, NP2, 2, CCOLS], F8)
                wql = p1w.tile([128, NP2, 2, CCOLS], F8)
                for pi in range(NP2):
                    nc.scalar.dma_start(wqh[:, pi], wq_hi[:, pi])
                    nc.scalar.dma_start(wql[:, pi], wq_lo[:, pi])
                nc.scalar.dma_start(cos_t[:], cos128[:])
                nc.scalar.dma_start(sin_t[:], snpm[:])
                nc.scalar.dma_start(mask_t[:], maskd[:])
                nc.scalar.dma_start(onec[:], ones_col[:])
                nc.scalar.dma_start(oner[:], ones_row[:].bitcast(F32R))
                for hp in range(2):
                    nc.scalar.dma_start(wd_hi_t[:, hp], wd_hi[:, hp])
                    nc.scalar.dma_start(wd_lo_t[:, hp], wd_lo[:, hp])

                for tcn in range(NCHUNK):
                    b = tcn // (SQ // CHUNK)
                    s0 = (tcn % (SQ // CHUNK)) * CHUNK
                    t0 = tcn * CHUNK
                    pss = [
                        p1ps.tile([128, CHUNK], F32, tag="qkvps",
                                  name=f"qkvps{ct}")
                        for ct in range(NCT)
                    ]
                    for pg in range(4):
                        hh = p1hid.tile([128, 4, 2, CHUNK], F8, tag="hh")
                        nc.sync.dma_start(
                            hh[:], hid_hi[:, pg * 4:(pg + 1) * 4, :, t0:t0 + CHUNK])
                        hl = p1hid.tile([128, 4, 2, CHUNK], F8, tag="hl")
                        nc.sync.dma_start(
                            hl[:], hid_lo[:, pg * 4:(pg + 1) * 4, :, t0:t0 + CHUNK])
                        for pl in range(4):
                            pi = pg * 4 + pl
                            for ct in range(NCT):
                                wh = wqh[:, pi, :, ct * 128:(ct + 1) * 128]
                                wl = wql[:, pi, :, ct * 128:(ct + 1) * 128]
                                nc.tensor.matmul(
                                    pss[ct][:], wh, hh[:, pl], perf_mode=DR,
                                    start=(pi == 0), stop=False)
                                nc.tensor.matmul(
                                    pss[ct][:], wl, hh[:, pl], perf_mode=DR,
                                    start=False, stop=False)
                                nc.tensor.matmul(
                                    pss[ct][:], wh, hl[:, pl], perf_mode=DR,
                                    start=False, stop=(pi == NP2 - 1))

                    cs = cos_t[:, s0:s0 + CHUNK]
                    sn = sin_t[:, s0:s0 + CHUNK]
                    # Q heads + K: bias, then rotary in bf16
                    for ct in range(HPC + 1):
                        mix = p1.tile([128, CHUNK], BF16, tag="mix")
                        nc.scalar.activation(
                            mix[:], pss[ct][:], AF.Identity,
                            bias=bq_t[:, ct:ct + 1], scale=1.0 / WSCALE)
                        if ct < HPC:
                            dst = qT[:, ct, t0:t0 + CHUNK]
                        else:
                            dst = kT[:, b, s0:s0 + CHUNK]
                        swp = p1.tile([64, CHUNK], BF16, tag="swp")
                        nc.sync.dma_start(swp[0:32], mix[32:64])
                        nc.sync.dma_start(swp[32:64], mix[0:32])
                        nc.vector.tensor_mul(out=dst, in0=mix[:], in1=cs)
                        nc.vector.tensor_mul(out=swp[:], in0=swp[:], in1=sn)
                        nc.vector.tensor_add(
                            out=dst[0:64], in0=dst[0:64], in1=swp[:])
                    # V: bias then transpose to token-major
                    mixv = p1.tile([128, CHUNK], BF16, tag="mixv")
                    nc.scalar.activation(
                        mixv[:], pss[HPC + 1][:], AF.Identity,
                        bias=bq_t[:, HPC + 1:HPC + 2], scale=1.0 / WSCALE)
                    for q4 in range(CHUNK // 128):
                        tps = p1tps.tile([128, 128], BF16, tag="tps")
                        nc.tensor.transpose(
                            tps[:], mixv[:, q4 * 128:(q4 + 1) * 128], ident[:])
                        nc.vector.tensor_copy(
                            out=v_tm[:, b, s0 // 128 + q4, :], in_=tps[:])

            # ---------- phase 2: attention + dense ----------
            with (
                tc.tile_pool(name="p2", bufs=4) as p2,
                tc.tile_pool(name="p2p", bufs=3) as p2p,
                tc.tile_pool(name="p2ctx", bufs=2) as p2ctx,
                tc.tile_pool(name="p2osb", bufs=2) as p2osb,
                tc.tile_pool(name="p2sps", bufs=2, space="PSUM") as p2sps,
                tc.tile_pool(name="p2cps", bufs=2, space="PSUM") as p2cps,
                tc.tile_pool(name="p2lps", bufs=1, space="PSUM") as p2lps,
                tc.tile_pool(name="p2bps", bufs=1, space="PSUM") as p2bps,
                tc.tile_pool(name="p2dps", bufs=2, space="PSUM") as p2dps,
            ):
                def emit_scale(linv, ctx_ps, h, chi, clo):
                    # 1/l broadcast + ctx scale + fp8 hi/lo split; emitted one
                    # head late so PE never waits on the DVE reciprocal
                    lb_ps = p2bps.tile([128, CHUNK], F32, tag="lbps")
                    nc.tensor.matmul(
                        lb_ps[:], oner[:], linv[:], start=True, stop=True)
                    lb_sb = p2.tile([128, CHUNK], F32, tag="lbsb")
                    nc.vector.tensor_copy(out=lb_sb[:], in_=lb_ps[:])
                    ctxT = p2.tile([128, CHUNK], F32, tag="ctxT")
                    nc.vector.tensor_mul(
                        out=ctxT[:], in0=ctx_ps[:], in1=lb_sb[:])
                    hp, hx = h // 2, h % 2
                    nc.scalar.activation(
                        chi[hp][:, hx, :], ctxT[:], AF.Identity)
                    nc.vector.tensor_sub(
                        out=clo[hp][:, hx, :], in0=ctxT[:],
                        in1=chi[hp][:, hx, :])

                for b in range(B):
                    for sc in range(SQ // CHUNK):
                        base = b * SQ + sc * CHUNK
                        n_t = (sc + 1) * (CHUNK // 128)
                        chi = [p2ctx.tile([128, 2, CHUNK], F8, tag="chi",
                                          name=f"chi{hp}") for hp in range(2)]
                        clo = [p2ctx.tile([128, 2, CHUNK], F8, tag="clo",
                                          name=f"clo{hp}") for hp in range(2)]
                        # two in-flight l vectors share one PSUM bank
                        # (partition rows 0 and 32)
                        lpair = p2lps.tile([33, CHUNK], F32, tag="lpair")
                        pend = None
                        for h in range(HPC):
                            ctx_ps = p2cps.tile([128, CHUNK], F32, tag="ctxps")
                            p_sum = p2p.tile([128, CHUNK], BF16, tag="psum")

                            def emit_ctx(p_r, so, tt):
                                nc.tensor.matmul(
                                    ctx_ps[:, so:], v_tm[:, b, tt, :],
                                    p_r[:, so:], start=(tt == 0),
                                    stop=(tt == n_t - 1),
                                    skip_group_check=True)

                            # PE order s(0), s(1), c(0), s(2), c(1), ... so
                            # the ctx matmul never waits on the exp latency
                            pend_ctx = None
                            for tt in range(n_t):
                                j = tt - sc * (CHUNK // 128)
                                so = 128 * j if j >= 0 else 0
                                sp = p2sps.tile([128, CHUNK], F32, tag="sps")
                                nc.tensor.matmul(
                                    sp[:, so:], kT[:, b, tt * 128:(tt + 1) * 128],
                                    qT[:, h, base + so:base + CHUNK],
                                    start=True, stop=True)
                                p_r = p2p.tile([128, CHUNK], BF16, tag="p")
                                nc.scalar.activation(
                                    p_r[:, so:], sp[:, so:], AF.Exp, scale=SCALE)
                                if j >= 0:
                                    nc.vector.tensor_mul(
                                        out=p_r[:, so:so + 128],
                                        in0=p_r[:, so:so + 128], in1=mask_t[:])
                                if tt == 0:
                                    nc.vector.tensor_copy(
                                        out=p_sum[:], in_=p_r[:])
                                else:
                                    nc.vector.tensor_add(
                                        out=p_sum[:, so:], in0=p_sum[:, so:],
                                        in1=p_r[:, so:])
                                if pend_ctx is not None:
                                    emit_ctx(*pend_ctx)
                                pend_ctx = (p_r, so, tt)
                            emit_ctx(*pend_ctx)
                            lrow = 32 * (h % 2)
                            l_out = lpair[lrow:lrow + 1, :]
                            nc.tensor.matmul(
                                l_out, onec[:], p_sum[:], start=True,
                                stop=True)
                            linv = p2.tile([1, CHUNK], F32R, tag="linv")
                            nc.vector.reciprocal(linv[:], l_out)
                            if pend is not None:
                                emit_scale(*pend, chi, clo)
                            pend = (linv, ctx_ps, h)
                        emit_scale(*pend, chi, clo)

                        row0 = b * SQ + sc * CHUNK
                        for st in range(CHUNK // 128):
                            osb = p2osb.tile([128, H // CHUNK, CHUNK], BF16,
                                             tag="osb")
                            for oc in range(H // CHUNK):
                                dps = p2dps.tile([128, CHUNK], F32, tag="dps")
                                for hp in range(2):
                                    cH = chi[hp][:, :, st * 128:(st + 1) * 128]
                                    cL = clo[hp][:, :, st * 128:(st + 1) * 128]
                                    wH = wd_hi_t[:, hp, :,
                                                 oc * CHUNK:(oc + 1) * CHUNK]
                                    wL = wd_lo_t[:, hp, :,
                                                 oc * CHUNK:(oc + 1) * CHUNK]
                                    nc.tensor.matmul(
                                        dps[:], cH, wH, perf_mode=DR,
                                        start=(hp == 0), stop=False)
                                    nc.tensor.matmul(
                                        dps[:], cH, wL, perf_mode=DR,
                                        start=False, stop=False)
                                    nc.tensor.matmul(
                                        dps[:], cL, wH, perf_mode=DR,
                                        start=False, stop=(hp == 1))
                                if oc % 2 == 0:
                                    nc.scalar.activation(
                                        osb[:, oc, :], dps[:], AF.Identity)
                                else:
                                    nc.vector.tensor_copy(
                                        out=osb[:, oc, :], in_=dps[:])
                            nc.sync.dma_start(
                                out_p[row0 + st * 128:row0 + (st + 1) * 128, :],
                                osb[:])

    nc.compile()
    return nc


def _split8(x):
    hi = x.astype(ml_dtypes.float8_e4m3)
    lo = (x - hi.astype(np.float32)).astype(ml_dtypes.float8_e4m3)
    return hi, lo


def _pair_layout(x, np2):
    # [np2*256, n] -> [128, np2, 2, n]
    n = x.shape[1]
    return np.ascontiguousarray(
        x.reshape(np2, 2, 128, n).transpose(2, 0, 1, 3))


def _host_inputs(hidden_states, rotary_pos_emb, W_qkv, b_qkv, W_dense):
    hidden_states = np.asarray(hidden_states, dtype=np.float32)
    rope = np.asarray(rotary_pos_emb, dtype=np.float32)
    W_qkv = np.asarray(W_qkv, dtype=np.float32)
    b_qkv = np.asarray(b_qkv, dtype=np.float32)
    W_dense = np.asarray(W_dense, dtype=np.float32)

    hidT = np.ascontiguousarray(
        hidden_states.transpose(2, 1, 0).reshape(H, TOK))
    h_hi, h_lo = _split8(hidT)
    hid_hi = _pair_layout(h_hi, NP2)
    hid_lo = _pair_layout(h_lo, NP2)

    cos = rope[:, :, 0].T  # [32, sq]
    sin = rope[:, :, 1].T
    cos128 = np.concatenate(
        [cos, cos, np.ones((64, SQ), np.float32)], axis=0
    ).astype(ml_dtypes.bfloat16)
    snpm = np.concatenate([-sin, sin], axis=0).astype(ml_dtypes.bfloat16)
    maskd = (np.arange(128)[None, :] >= np.arange(128)[:, None]).astype(
        ml_dtypes.bfloat16)
    ones_col = np.ones((128, 1), ml_dtypes.bfloat16)
    ones_row = np.ones((1, 128), np.float32)

    perm = np.concatenate(
        [np.arange(0, ROT, 2), np.arange(1, ROT, 2), np.arange(ROT, HD)])
    in_maps = []
    for c in range(N_CORES):
        g = c // (N_CORES // NG)
        qcols = [h * HD + perm for h in range(HPC * c, HPC * (c + 1))]
        kcols = NH * HD + g * HD + perm
        vcols = NH * HD + NG * HD + g * HD + np.arange(HD)
        cols = np.concatenate(qcols + [kcols, vcols])
        wq_c = np.ascontiguousarray(W_qkv[:, cols]) * WSCALE
        w_hi, w_lo = _split8(wq_c)
        bq_c = np.ascontiguousarray(b_qkv[cols].reshape(NCT, 128).T)
        wd_c = np.ascontiguousarray(W_dense[c * QCOLS:(c + 1) * QCOLS, :]) * WSCALE
        d_hi, d_lo = _split8(wd_c)
        in_maps.append({
            "hid_hi": hid_hi, "hid_lo": hid_lo,
            "wq_hi": _pair_layout(w_hi, NP2), "wq_lo": _pair_layout(w_lo, NP2),
            "bq": bq_c,
            "wd_hi": _pair_layout(d_hi, 2), "wd_lo": _pair_layout(d_lo, 2),
            "cos128": cos128, "snpm": snpm, "maskd": maskd,
            "ones_col": ones_col, "ones_row": ones_row,
        })
    return in_maps


def kernel(hidden_states, attention_mask, rotary_pos_emb, W_qkv, b_qkv,
           W_dense, _trace=False):
    if "nc" not in _CACHE:
        _CACHE["nc"] = _build()
    nc = _CACHE["nc"]
    in_maps = _host_inputs(
        hidden_states, rotary_pos_emb, W_qkv, b_qkv, W_dense)
    res = run_bass_kernel_spmd(
        nc, in_maps, list(range(N_CORES)), trace=_trace)
    acc = res.results[0]["out_p"].astype(np.float32)
    for c in range(1, N_CORES):
        acc += res.results[c]["out_p"].astype(np.float32)
    acc *= 1.0 / WSCALE
    out = acc.reshape(B, SQ, H).transpose(1, 0, 2)
    out = np.ascontiguousarray(out)
    _CACHE["last_result"] = res
    return out
